# revision 17
# baseline (speedup 1.0000x reference)
"""Multi-head attention (B=2, S=2048, D=1024, H=16) on 8 NeuronCores.

Sharding: data-parallel over batch (2) x tensor-parallel over head groups (4).
Core c handles batch c//4, heads [4*(c%4), 4*(c%4)+4).

Per-core device kernel (Bass/Tile, fp32 storage, f32r matmuls):
  QT = Wq_g @ x_q.T         [256, 2048]   (head-transposed layout)
  KT = Wk_g @ x_k.T         [256, 2048]
  V  = x_v @ Wv_g.T         [2048, 256]
  per head h (64 dims):
    H1: scores[q,k] tiles -> exp (ACT, scale=1/8, fused row-sum) -> normalize
        (DVE, per-partition 1/sum) -> DMA out attn[h]
    H2: scores[k,q] tiles (recomputed transposed; softmax transpose is
        cheaper on PE as a second gemm than any transpose path) -> exp ->
        ctxT[h] += V_h.T-free gemm accumulated in PSUM over k chunks
    ctxT evict: multiply by 1/sums broadcast along free dim (round-tripped
        through DRAM to convert partition-layout sums to free-layout)
  pout = ctxT.T @ Wo_g.T    [2048, 1024]  (partial; host sums the 4 groups)

Host: transposes inputs once per batch, slices weights per group, gathers
attn parts and sums pout partials.
"""
import sys

sys.path.insert(0, "/opt/trn_rl_repo")

import numpy as np
from contextlib import ExitStack

import concourse.bass as bass
import concourse.mybir as mybir
import concourse.tile as tile
from concourse import bacc, bass_utils

F32 = mybir.dt.float32
F32R = mybir.dt.float32r
EXP = mybir.ActivationFunctionType.Exp
MULT = mybir.AluOpType.mult

B, S, D, H = 2, 2048, 1024, 16
P = 128
NH = 4          # heads per core
DH = 64         # head dim
HD = NH * DH    # 256, per-core projection width
KC = D // P     # 8 contraction chunks for projections
SC = S // P     # 16 sequence chunks
SCALE = 1.0 / np.sqrt(np.float32(DH))  # 0.125


def build_nc(parts=("proj", "h1", "h2", "pout")):
    nc = bacc.Bacc("TRN2", target_bir_lowering=False, debug=False)

    xqT = nc.dram_tensor("xqT", [D, S], F32R, kind="ExternalInput").ap()
    xkT = nc.dram_tensor("xkT", [D, S], F32R, kind="ExternalInput").ap()
    xvT = nc.dram_tensor("xvT", [D, S], F32R, kind="ExternalInput").ap()
    wqT = nc.dram_tensor("wqT", [D, HD], F32R, kind="ExternalInput").ap()
    wkT = nc.dram_tensor("wkT", [D, HD], F32R, kind="ExternalInput").ap()
    wvT = nc.dram_tensor("wvT", [D, HD], F32R, kind="ExternalInput").ap()
    woT = nc.dram_tensor("woT", [HD, D], F32R, kind="ExternalInput").ap()
    attn_o = nc.dram_tensor("attn_o", [NH, S, S], F32, kind="ExternalOutput").ap()
    pout_o = nc.dram_tensor("pout_o", [S, D], F32, kind="ExternalOutput").ap()
    inv_o = nc.dram_tensor("inv_o", [NH, P, SC], F32, kind="ExternalOutput").ap()

    with tile.TileContext(nc) as tc:
        with ExitStack() as ctx:
            wpool = ctx.enter_context(tc.tile_pool(name="wpool", bufs=1))
            xpool = ctx.enter_context(tc.tile_pool(name="xpool", bufs=4))
            persist = ctx.enter_context(tc.tile_pool(name="persist", bufs=1))
            atpool = ctx.enter_context(tc.tile_pool(name="atpool", bufs=4))
            etpool = ctx.enter_context(tc.tile_pool(name="etpool", bufs=3))
            smpool = ctx.enter_context(tc.tile_pool(name="smpool", bufs=4))
            ifpool = ctx.enter_context(tc.tile_pool(name="ifpool", bufs=2))
            ps = ctx.enter_context(tc.tile_pool(name="ps", bufs=2, space="PSUM"))
            cxps = ctx.enter_context(tc.tile_pool(name="cxps", bufs=1, space="PSUM"))
            dr = ctx.enter_context(tc.tile_pool(name="dr", bufs=2, space="DRAM"))

            # --- weights to SBUF ---
            wq_s = wpool.tile([P, KC, HD], F32R, tag="wq")
            wk_s = wpool.tile([P, KC, HD], F32R, tag="wk")
            wv_s = wpool.tile([P, KC, HD], F32R, tag="wv")
            wo_s = wpool.tile([P, HD // P, D], F32R, tag="wo")
            nc.sync.dma_start(wq_s, wqT.rearrange("(c p) n -> p c n", p=P))
            nc.sync.dma_start(wk_s, wkT.rearrange("(c p) n -> p c n", p=P))
            nc.sync.dma_start(wv_s, wvT.rearrange("(c p) n -> p c n", p=P))
            nc.sync.dma_start(wo_s, woT.rearrange("(c p) n -> p c n", p=P))

            QT_s = persist.tile([P, 2, S], F32R, tag="QT")
            KT_s = persist.tile([P, 2, S], F32R, tag="KT")
            V_s = persist.tile([P, SC, HD], F32R, tag="V")
            ctxT_s = persist.tile([P, 2, S], F32R, tag="ctxT")
            invs = [persist.tile([P, SC], F32, tag=f"invs{h}", name=f"invs{h}") for h in range(NH)]

            # --- projections: QT/KT = W @ x.T, accumulated in SBUF over KC ---
            for xT, w_s, out_s in (((xqT, wq_s, QT_s), (xkT, wk_s, KT_s)) if "proj" in parts else ()):
                for kcp in range(KC // 2):
                    xcs = []
                    for j in range(2):
                        xc = xpool.tile([P, S], F32R, tag="xc", name=f"xc{j}")
                        nc.sync.dma_start(xc, xT[(2 * kcp + j) * P:(2 * kcp + j + 1) * P, :])
                        xcs.append(xc)
                    for m in range(2):
                        for ns in range(4):
                            pt = cxps.tile([P, 512], F32, tag=f"cx{(m * 4 + ns) % 4}",
                                           name=f"pj{m}{ns}")
                            for j in range(2):
                                nc.tensor.matmul(
                                    pt,
                                    lhsT=w_s[:, 2 * kcp + j, m * P:(m + 1) * P],
                                    rhs=xcs[j][:, ns * 512:(ns + 1) * 512],
                                    start=(j == 0), stop=(j == 1),
                                )
                            dst = out_s[:, m, ns * 512:(ns + 1) * 512]
                            if kcp == 0:
                                nc.vector.tensor_copy(out=dst, in_=pt)
                            else:
                                nc.vector.tensor_add(out=dst, in0=dst, in1=pt)

            # --- V = x_v @ Wv.T  [2048, 256], SBUF-accumulated ---
            for kcp in (range(KC // 2) if "proj" in parts else ()):
                xcs = []
                for j in range(2):
                    xc = xpool.tile([P, S], F32R, tag="xc", name=f"xcv{j}")
                    nc.sync.dma_start(xc, xvT[(2 * kcp + j) * P:(2 * kcp + j + 1) * P, :])
                    xcs.append(xc)
                for sc in range(SC):
                    pt = cxps.tile([P, HD], F32, tag=f"cx{sc % 4}", name=f"pjv{sc % 4}")
                    for j in range(2):
                        nc.tensor.matmul(
                            pt,
                            lhsT=xcs[j][:, sc * P:(sc + 1) * P],
                            rhs=wv_s[:, 2 * kcp + j, :],
                            start=(j == 0), stop=(j == 1),
                        )
                    dst = V_s[:, sc, :]
                    if kcp == 0:
                        nc.vector.tensor_copy(out=dst, in_=pt)
                    else:
                        nc.vector.tensor_add(out=dst, in0=dst, in1=pt)

            # --- attention: H1(h) interleaved with H2(h-1) (one-head lag) ---
            do_h1 = "h1" in parts
            do_h2 = "h2" in parts

            def h2_iter(h, kc, cx):
                hp = (h % 2) * DH
                hc = h // 2
                QT_h = QT_s[hp:hp + DH, hc, :]
                KT_h = KT_s[hp:hp + DH, hc, :]
                et = etpool.tile([P, S], F32R, tag="et", name=f"et{h}_{kc}")
                for half in range(2):
                    pt = ps.tile([P, 1024], F32, tag="sc", name=f"p2_{h}_{kc}_{half}")
                    for j in range(2):
                        nsl = half * 2 + j
                        nc.tensor.matmul(
                            pt[:, j * 512:(j + 1) * 512],
                            lhsT=KT_h[:, kc * P:(kc + 1) * P],
                            rhs=QT_h[:, nsl * 512:(nsl + 1) * 512],
                            start=True, stop=True,
                        )
                    nc.scalar.activation(
                        et[:, half * 1024:(half + 1) * 1024], pt, EXP,
                        scale=float(SCALE),
                    )
                for ns in range(4):
                    nc.tensor.matmul(
                        cx[ns],
                        lhsT=V_s[:, kc, h * DH:(h + 1) * DH],
                        rhs=et[:, ns * 512:(ns + 1) * 512],
                        start=(kc == 0), stop=(kc == SC - 1),
                    )

            def h1_iter(h, qc):
                hp = (h % 2) * DH
                hc = h // 2
                QT_h = QT_s[hp:hp + DH, hc, :]
                KT_h = KT_s[hp:hp + DH, hc, :]
                at = atpool.tile([P, S], F32, tag="at", name=f"at{h}_{qc}")
                pa = smpool.tile([P, 1], F32, tag="pa", name=f"pa{h}_{qc}")
                pb = smpool.tile([P, 1], F32, tag="pb", name=f"pb{h}_{qc}")
                for half in range(2):
                    pt = ps.tile([P, 1024], F32, tag="sc", name=f"p1_{h}_{qc}_{half}")
                    for j in range(2):
                        nsl = half * 2 + j
                        nc.tensor.matmul(
                            pt[:, j * 512:(j + 1) * 512],
                            lhsT=QT_h[:, qc * P:(qc + 1) * P],
                            rhs=KT_h[:, nsl * 512:(nsl + 1) * 512],
                            start=True, stop=True,
                        )
                    nc.scalar.activation(
                        at[:, half * 1024:(half + 1) * 1024], pt, EXP,
                        scale=float(SCALE),
                        accum_out=(pa if half == 0 else pb),
                    )
                nc.sync.dma_start(attn_o[h, qc * P:(qc + 1) * P, :], at)
                sums = smpool.tile([P, 1], F32, tag="sums", name=f"sm{h}_{qc}")
                nc.vector.tensor_add(out=sums, in0=pa, in1=pb)
                nc.vector.reciprocal(invs[h][:, qc:qc + 1], sums)

            def inv_roundtrip(h):
                nc.sync.dma_start(inv_o[h], invs[h])
                dsc = dr.tile([SC, P], F32, tag="dsc", name=f"dsc{h}")
                nc.sync.dma_start(dsc.rearrange("c p -> p c"), invs[h])
                inv_b = ifpool.tile([DH, S], F32, tag="invb", name=f"invb{h}")
                nc.sync.dma_start(
                    inv_b,
                    dsc.rearrange("c p -> (c p)")[None, :].to_broadcast([DH, S]),
                )
                return inv_b

            def evict(h, cx, inv_b):
                hp = (h % 2) * DH
                hc = h // 2
                for ns in range(4):
                    nc.vector.tensor_tensor(
                        ctxT_s[hp:hp + DH, hc, ns * 512:(ns + 1) * 512],
                        cx[ns],
                        inv_b[:, ns * 512:(ns + 1) * 512],
                        MULT,
                    )

            pos = {}

            def pout_piece(m, ns):
                # pout[m, ns] = (heads 0-2 normalized ctxT) @ wo
                #             + inv3[q] * (head-3 unnormalized ctxT) @ wo
                pt = ps.tile([P, 512], F32, tag="sc", name=f"pop{m}_{ns}")
                nc.tensor.matmul(
                    pt,
                    lhsT=ctxT_s[:, 0, m * P:(m + 1) * P],
                    rhs=wo_s[:, 0, ns * 512:(ns + 1) * 512],
                    start=True, stop=False,
                )
                nc.tensor.matmul(
                    pt,
                    lhsT=ctxT_s[0:DH, 1, m * P:(m + 1) * P],
                    rhs=wo_s[0:DH, 1, ns * 512:(ns + 1) * 512],
                    start=False, stop=True,
                )
                pt3 = cxps.tile([P, 512], F32, tag=f"cx{(2 * m + ns) % 4}",
                                name=f"pq{m}_{ns}")
                nc.tensor.matmul(
                    pt3,
                    lhsT=ctxT_s[DH:P, 1, m * P:(m + 1) * P],
                    rhs=wo_s[DH:P, 1, ns * 512:(ns + 1) * 512],
                    start=True, stop=True,
                )
                po = pos[m]
                ph = po[:, ns * 512:(ns + 1) * 512]
                nc.vector.tensor_copy(out=ph, in_=pt)
                nc.vector.scalar_tensor_tensor(
                    ph, pt3, invs[NH - 1][:, m:m + 1], ph,
                    MULT, mybir.AluOpType.add,
                )
                if ns == 1:
                    nc.sync.dma_start(pout_o[m * P:(m + 1) * P, :], po)

            if do_h1 or do_h2:
                inv_bs = {}
                cxs = {}
                do_po = "pout" in parts and do_h1 and do_h2
                for h in range(NH):
                    if do_h2 and h > 0:
                        cxs[h - 1] = [cxps.tile([DH, 512], F32, tag=f"cx{ns}",
                                                name=f"cx{h-1}_{ns}")
                                      for ns in range(4)]
                    for i in range(SC):
                        if do_h2 and h > 0:
                            h2_iter(h - 1, i, cxs[h - 1])
                        if do_h1:
                            h1_iter(h, i)
                    if h < NH - 1:
                        inv_bs[h] = inv_roundtrip(h)
                    elif do_h1:
                        nc.sync.dma_start(inv_o[h], invs[h])
                    if do_h2 and h > 0:
                        evict(h - 1, cxs[h - 1], inv_bs[h - 1])
                if do_h2:
                    h = NH - 1
                    cxs[h] = [cxps.tile([DH, 512], F32, tag=f"cx{ns}",
                                        name=f"cx{h}_{ns}") for ns in range(4)]
                    for i in range(SC):
                        h2_iter(h, i, cxs[h])
                    # evict head 3 unnormalized (plain copy); inv applied in pout
                    for ns in range(4):
                        nc.vector.tensor_copy(
                            out=ctxT_s[DH:P, 1, ns * 512:(ns + 1) * 512],
                            in_=cxs[h][ns])
                    if do_po:
                        for m in range(SC):
                            pos[m] = atpool.tile([P, D], F32, tag="at",
                                                 name=f"po{m}")
                            for ns in range(2):
                                pout_piece(m, ns)

    nc.compile()
    return nc


_CACHE = {}


def _get_nc():
    if "nc" not in _CACHE:
        _CACHE["nc"] = build_nc()
    return _CACHE["nc"]


def _numpy_fallback(query, key, value, mask, Wq, Wk, Wv, Wo):
    DK = D // H
    Q = (query @ Wq.T).reshape(B, S, H, DK).transpose(0, 2, 1, 3)
    K = (key @ Wk.T).reshape(B, -1, H, DK).transpose(0, 2, 1, 3)
    V = (value @ Wv.T).reshape(B, -1, H, DK).transpose(0, 2, 1, 3)
    scores = np.einsum("bhqd,bhkd->bhqk", Q, K) / np.sqrt(np.float32(DK))
    scores = np.where(mask == 0, np.float32(-1e9), scores)
    scores -= scores.max(axis=-1, keepdims=True)
    attn = np.exp(scores)
    attn /= attn.sum(axis=-1, keepdims=True)
    out = np.einsum("bhqk,bhkd->bhqd", attn, V)
    out = out.transpose(0, 2, 1, 3).reshape(B, S, D)
    return (out @ Wo.T).astype(np.float32), attn.astype(np.float32)


def kernel(query, key, value, mask, Wq, Wk, Wv, Wo):
    query = np.asarray(query, dtype=np.float32)
    key = np.asarray(key, dtype=np.float32)
    value = np.asarray(value, dtype=np.float32)
    mask = np.asarray(mask)
    Wq = np.asarray(Wq, dtype=np.float32)
    Wk = np.asarray(Wk, dtype=np.float32)
    Wv = np.asarray(Wv, dtype=np.float32)
    Wo = np.asarray(Wo, dtype=np.float32)

    if not np.all(mask == 1):
        return _numpy_fallback(query, key, value, mask, Wq, Wk, Wv, Wo)

    qT = [np.ascontiguousarray(query[b].T) for b in range(B)]
    kT = [np.ascontiguousarray(key[b].T) for b in range(B)]
    vT = [np.ascontiguousarray(value[b].T) for b in range(B)]

    in_maps = []
    for c in range(8):
        b, g = divmod(c, 4)
        gs = slice(g * HD, (g + 1) * HD)
        in_maps.append({
            "xqT": qT[b], "xkT": kT[b], "xvT": vT[b],
            "wqT": np.ascontiguousarray(Wq[gs].T),
            "wkT": np.ascontiguousarray(Wk[gs].T),
            "wvT": np.ascontiguousarray(Wv[gs].T),
            "woT": np.ascontiguousarray(Wo[:, gs].T),
        })

    nc = _get_nc()
    res = bass_utils.run_bass_kernel_spmd(nc, in_maps, core_ids=list(range(8)))

    attn = np.empty((B, H, S, S), dtype=np.float32)
    output = np.zeros((B, S, D), dtype=np.float32)
    for c in range(8):
        b, g = divmod(c, 4)
        part = res.results[c]["attn_o"]           # [NH, S, S] unnormalized
        inv = res.results[c]["inv_o"]             # [NH, P, SC]; q = qc*P + p
        inv_q = inv.transpose(0, 2, 1).reshape(NH, S, 1)
        np.multiply(part, inv_q, out=attn[b, g * NH:(g + 1) * NH])
        output[b] += res.results[c]["pout_o"]
    return output, attn
